# revision 26
# baseline (speedup 1.0000x reference)
"""Self-contained TRN2 Bass kernel for nn_EnhancedMultiheadAttention.

kernel(**inputs) takes the FULL unsharded inputs (x, Wq, bq, Wk, bk, Wv, bv,
Wo, bo as float32 numpy arrays), distributes the computation across 8
NeuronCores (tensor-parallel over heads: core c owns heads 2c, 2c+1), and
returns the full [2, 2048, 1024] float32 output.

v2 design notes (vs the original baseline):
- all operand storage is bf16 (fp32 accumulation everywhere): halves the
  x-stream DMA (the phase-A bottleneck), SBUF footprint, and A2A payload.
- emission is interleaved so attention on batch 0 starts while batch 1 of
  x is still streaming in; all x chunk loads are enqueued up-front on the
  sync queue so no compute-dependent DMA blocks them (FIFO queues).
- V is transposed with the DMA XBAR (dma_start_transpose on the scalar
  queue) instead of PE transposes: frees PE time and 2 PSUM banks.
- softmax reciprocal uses reciprocal_approx_fast (~5x faster than the
  6.5us/call InstReciprocal the baseline used).
- bv folds into bo on the host (softmax weights sum to 1), so V needs no
  bias ACT; bq/bk stay as ACT bias adds.
- one AllToAll per head, doorbelled as soon as that head's context is
  written; the a2a_out gathers run on the sync queue so they cannot block
  the second collective's trigger (the baseline lost ~37us to that).
- Wo rows are permuted on the host so phase C can accumulate head-0's
  contribution while head-1's A2A is still in flight (per-head feature
  chunks), with the 8 output accumulators held in PSUM across the wait.
"""

import sys

for _p in ("/opt/trn_rl_repo", "/root/.axon_site/_ro/trn_rl_repo"):
    if _p not in sys.path:
        sys.path.append(_p)


import numpy as np

import concourse.bass as bass
import concourse.mybir as mybir
import concourse.tile as tile
import bass_rust

F32 = mybir.dt.float32
BF16 = mybir.dt.bfloat16

B, L, D = 2, 2048, 1024
H, DH = 16, 64
NCORES = 8
T = B * L                  # 4096 flattened tokens
TC = T // NCORES           # 512 tokens per core for the output slice
NKC = D // 128             # 8 contraction chunks of 128
CHUNK = 1024               # projection token-chunk width
NT = T // CHUNK            # 4 projection chunks
NJ = L // 128              # 16 key chunks of 128 per sequence
IB = 1024                  # query block width
NI = L // IB               # 2 query blocks per sequence
HPC = H // NCORES          # 2 heads per core


def split_excess_waits(nc, max_waits=1):
    """walrus's setupSyncWait rejects instructions with more than one wait
    condition on this compiler version; hoist extras onto preceding NoOps."""
    n_split = 0
    for f in nc.m.functions:
        for b in f.blocks:
            new_list = None
            for inst in list(b.instructions):
                si = inst.sync_info
                if si is None or len(si.on_wait) <= max_waits:
                    continue
                waits = list(si.on_wait)
                keep = waits[-max_waits:]
                excess = waits[:-max_waits]
                nops = []
                for j, w in enumerate(excess):
                    nop = mybir.InstNoOp(
                        name=f"I-wsplit-{inst.name}-{j}", ins=[], outs=[],
                        engine=inst.engine,
                    )
                    nop.sync_info = bass_rust.SyncInfo(on_wait=[w], on_update=[])
                    nops.append(nop)
                inst.sync_info = bass_rust.SyncInfo(
                    on_wait=keep, on_update=list(si.on_update)
                )
                if new_list is None:
                    new_list = list(b.instructions)
                pos = new_list.index(inst)
                new_list[pos:pos] = nops
                n_split += 1
            if new_list is not None:
                b.instructions = new_list
    return n_split


def build_nc(**_unused_mm_kwargs):
    nc = bass.Bass("TRN2", target_bir_lowering=False, debug=False,
                   num_devices=NCORES)

    xT = nc.dram_tensor("xT", [D, T], BF16, kind="ExternalInput").ap()
    wq = nc.dram_tensor("wq", [D, 128], BF16, kind="ExternalInput").ap()
    wk = nc.dram_tensor("wk", [D, 128], BF16, kind="ExternalInput").ap()
    wv = nc.dram_tensor("wv", [D, 128], BF16, kind="ExternalInput").ap()
    bq = nc.dram_tensor("bq", [128, 1], F32, kind="ExternalInput").ap()
    bk = nc.dram_tensor("bk", [128, 1], F32, kind="ExternalInput").ap()
    wo = nc.dram_tensor("wo", [D, D], BF16, kind="ExternalInput").ap()
    bo = nc.dram_tensor("bo", [1, D], F32, kind="ExternalInput").ap()
    cosT = nc.dram_tensor("cosT", [128, L], BF16, kind="ExternalInput").ap()
    sinT = nc.dram_tensor("sinT", [128, L], BF16, kind="ExternalInput").ap()
    out = nc.dram_tensor("out", [TC, D], F32, kind="ExternalOutput").ap()

    with tile.TileContext(nc) as tc:
        _build_body(nc, tc, xT, wq, wk, wv, bq, bk, wo, bo, cosT, sinT, out)

    split_excess_waits(nc)
    return nc


def _build_body(nc, tc, xT, wq, wk, wv, bq, bk, wo, bo, cosT, sinT, out):
    from contextlib import ExitStack

    ctx = ExitStack()
    with ctx, nc.allow_low_precision(reason="bf16 operand storage by design"):
        # ---------------- persistent tensors ----------------
        persist = ctx.enter_context(tc.tile_pool(name="persist", bufs=1))
        qt_sb = persist.tile([128, T], BF16, tag="qt", name="qt")
        kt_sb = [persist.tile([128, T], BF16, tag=f"kt{h}", name=f"kt{h}")
                 for h in range(HPC)]
        v_sb = [persist.tile([128, T // 128, DH + 1], BF16, tag=f"v{h}",
                             name=f"v{h}")
                for h in range(HPC)]

        # ones column of V (softmax denominator trick) + zero pads of K
        ones_col = persist.tile([128, 1], F32, tag="ones", name="ones")
        nc.gpsimd.memset(ones_col[:], 1.0)
        for h in range(HPC):
            nc.vector.tensor_copy(
                v_sb[h][:, :, DH:DH + 1],
                ones_col[:, :].to_broadcast((128, T // 128, 1)))
        zero_col = persist.tile([128, 1], F32, tag="zeros", name="zeros")
        nc.gpsimd.memset(zero_col[:], 0.0)
        nc.vector.tensor_copy(kt_sb[0][DH:128, :],
                              zero_col[DH:128, :].to_broadcast((DH, T)))
        nc.vector.tensor_copy(kt_sb[1][0:DH, :],
                              zero_col[0:DH, :].to_broadcast((DH, T)))

        wpool = ctx.enter_context(tc.tile_pool(name="wqkv", bufs=1))
        w_t = {}
        b_t = {}
        for name, wap, bap in (("q", wq, bq), ("k", wk, bk), ("v", wv, None)):
            w_t[name] = wpool.tile([128, NKC, 128], BF16, tag=f"w{name}",
                                   name=f"w{name}")
            nc.sync.dma_start(w_t[name][:],
                              wap.rearrange("(kc p) m -> p kc m", p=128))
            if bap is not None:
                b_t[name] = wpool.tile([128, 1], F32, tag=f"b{name}",
                                       name=f"b{name}")
                nc.sync.dma_start(b_t[name][:], bap)

        # DRAM buffers for the collectives (one per local head).  Each of
        # the 8 blocks is 65 rows: 64 unnormalized context rows plus the
        # softmax denominator row, so normalization can happen on the
        # receiving side (off the pre-doorbell critical path).
        BR = DH + 1
        dram = ctx.enter_context(tc.tile_pool(name="dram", bufs=1, space="DRAM"))
        a2a_in = [dram.tile([NCORES * BR, TC], BF16, name=f"a2a_in{h}")
                  for h in range(HPC)]
        a2a_out = [dram.tile([NCORES * BR, TC], BF16, name=f"a2a_out{h}")
                   for h in range(HPC)]
        r16d = dram.tile([HPC, NCORES, TC], F32, name="r16d")

        xT3 = xT.rearrange("(kc p) t -> p kc t", p=128)

        # ---------------- pools shared across phases A/B ----------------
        xpool = ctx.enter_context(tc.tile_pool(name="x", bufs=NT))
        cspool = ctx.enter_context(tc.tile_pool(name="cs", bufs=1))

        cos_l = cspool.tile([128, L], BF16, tag="cos", name="cos")
        sin_l = cspool.tile([128, L], BF16, tag="sin", name="sin")
        nc.sync.dma_start(cos_l[:], cosT[:])
        nc.sync.dma_start(sin_l[:], sinT[:])

        # opool must sit on the outer stack so it survives into phase C.
        opool = ctx.enter_context(tc.tile_pool(name="oproj", bufs=1))
        wo_sb = opool.tile([128, NKC, D], BF16, tag="wo", name="wo")
        bo_sb = opool.tile([128, D], F32, tag="bo", name="bo")
        ctx_sb = opool.tile([128, NKC, TC], BF16, tag="ctxsb", name="ctxsb")
        # post-A2A normalization working tiles (allocated up-front so they
        # can be dependency-touched at phase-B end; see markers below)
        ld64 = opool.tile([64, HPC, 64], BF16, tag="ld64", name="ld64")
        rl64 = opool.tile([64, HPC, 64], F32, tag="rl64", name="rl64")
        rl_sb = opool.tile([128, NKC, TC], F32, tag="rlsb", name="rlsb")

        # phase A/B pools, closed before phase C so the 8 PSUM banks used
        # by bigpsum+avpsum free up for the 8 output accumulators.
        abctx = ExitStack()
        tmp = abctx.enter_context(tc.tile_pool(name="ptmp", bufs=3))
        # 'big' psum tag is shared by projection accumulators, V-transpose
        # tiles, score tiles and the 1/l broadcast: [128,1024]f32 = 2
        # banks, bufs=2 -> 4 banks
        bigpsum = abctx.enter_context(tc.tile_pool(name="bigpsum", bufs=2,
                                                   space="PSUM"))
        avpsum = abctx.enter_context(tc.tile_pool(name="avpsum", bufs=2,
                                                  space="PSUM"))
        ppool = abctx.enter_context(tc.tile_pool(name="pT", bufs=4))
        npool = abctx.enter_context(tc.tile_pool(name="norm", bufs=2))
        cpool = abctx.enter_context(tc.tile_pool(name="ctx", bufs=2))
        ndram = abctx.enter_context(tc.tile_pool(name="ndram", bufs=2,
                                                 space="DRAM"))

        # ---------- phase A pieces ----------
        x_tiles = {}

        def load_x_chunk(i):
            tsl = bass.ts(i, CHUNK)
            xt = xpool.tile([128, NKC, CHUNK], BF16, tag="xchunk",
                            name=f"xchunk{i}")
            for kc in range(NKC):
                nc.sync.dma_start(xt[:, kc, :], xT3[:, kc, tsl])
            x_tiles[i] = xt

        def proj_qk(i, name):
            """q or k projection + RoPE for token chunk i (one work unit)."""
            xt = x_tiles[i]
            tsl = bass.ts(i, CHUNK)
            lsl = bass.ts(i % (L // CHUNK), CHUNK)
            cos_t = cos_l[:, lsl]
            sin_t = sin_l[:, lsl]

            ps = bigpsum.tile([128, CHUNK], F32, tag="big", name="proj")
            for kc in range(NKC):
                for nh in range(CHUNK // 512):
                    nc.tensor.matmul(
                        ps[:, bass.ts(nh, 512)],
                        w_t[name][:, kc, :],
                        xt[:, kc, bass.ts(nh, 512)],
                        start=(kc == 0), stop=(kc == NKC - 1),
                    )
            # bias on DVE (keeps the scalar engine free for exps)
            raw = tmp.tile([128, CHUNK], BF16, tag="raw", name="raw")
            nc.vector.tensor_scalar_add(raw[:], ps[:], b_t[name][:])
            shifted = tmp.tile([128, CHUNK], BF16, tag="shift", name="shift")
            for h in range(HPC):
                o = h * DH
                nc.gpsimd.dma_start(shifted[o:o + 32, :],
                                    raw[o + 32:o + 64, :])
                nc.gpsimd.dma_start(shifted[o + 32:o + 64, :],
                                    raw[o:o + 32, :])
            t1 = tmp.tile([128, CHUNK], BF16, tag="t1", name="t1")
            nc.vector.tensor_mul(t1[:], raw[:], cos_t)
            nc.vector.tensor_mul(shifted[:], shifted[:], sin_t)
            if name == "q":
                nc.vector.tensor_add(qt_sb[:, tsl], t1[:], shifted[:])
            else:
                for h in range(HPC):
                    o = h * DH
                    nc.vector.tensor_add(
                        kt_sb[h][o:o + DH, tsl],
                        t1[o:o + DH, :], shifted[o:o + DH, :])

        def proj_v_pair(i, pair):
            """V projection for token blocks 2*pair, 2*pair+1 of chunk i,
            directly in [token, dh] orientation (stationary = x block,
            moving = Wv) so no transpose is needed.  Lives in the "big"
            psum ring, which the PE stream itself drains.  No V bias
            (folded into bo on the host)."""
            xt = x_tiles[i]
            vt = bigpsum.tile([128, CHUNK], F32, tag="big", name="vt")
            for sub in range(2):
                blk = pair * 2 + sub
                for kc in range(NKC):
                    nc.tensor.matmul(
                        vt[:, bass.ts(sub, 128)],
                        xt[:, kc, bass.ds(blk * 128, 128)],
                        w_t["v"][:, kc, :],
                        start=(kc == 0), stop=(kc == NKC - 1),
                    )
                jg = i * (CHUNK // 128) + blk
                for h in range(HPC):
                    nc.vector.tensor_copy(
                        v_sb[h][:, jg, 0:DH],
                        vt[:, bass.ds(sub * 128 + h * DH, DH)])

        def proj_units(i, order="kvq"):
            """generator of ~2-4us PE work units for chunk i's projections,
            so emission can interleave them with attention iterations."""
            for c in order:
                if c == "q":
                    proj_qk(i, "q")
                    yield
                elif c == "k":
                    proj_qk(i, "k")
                    yield
                else:
                    for pair in range(CHUNK // 256):
                        proj_v_pair(i, pair)
                        yield

        def proj_chunk(i, order="kvq"):
            for _ in proj_units(i, order):
                pass

        # ---------- phase B pieces ----------
        def attn_jc(h, b, ib, jc, av, filler=None):
            co = b * L
            st = bigpsum.tile([128, IB], F32, tag="big", name="st")
            for nh in range(IB // 512):
                nc.tensor.matmul(
                    st[:, bass.ts(nh, 512)],
                    kt_sb[h][:, bass.ds(co + jc * 128, 128)],
                    qt_sb[:, bass.ds(co + ib * IB + nh * 512, 512)],
                    start=True, stop=True,
                )
            pt = ppool.tile([128, IB], BF16, tag="pt", name="pt")
            nc.scalar.activation(pt[:], st[:],
                                 mybir.ActivationFunctionType.Exp,
                                 scale=float(DH) ** -0.5)
            if filler is not None:
                # filler PE work lands between the score and AV matmuls,
                # exactly where the PE would otherwise wait on the exp.
                next(filler, None)
            for nh in range(IB // 512):
                nc.tensor.matmul(
                    av[0:DH + 1, bass.ts(nh, 512)],
                    v_sb[h][:, b * NJ + jc, :],
                    pt[:, bass.ts(nh, 512)],
                    start=(jc == 0), stop=(jc == NJ - 1),
                )

        def attn_finish_ib(h, b, ib, av):
            # evacuate the unnormalized context + denominator row; the
            # normalization happens on the receiving side of the A2A.
            cxl = cpool.tile([DH + 1, IB], BF16, tag="cx", name="cx")
            nc.vector.tensor_copy(cxl[:], av[0:DH + 1, :])
            for half in range(IB // TC):
                g = (b * L + ib * IB + half * TC) // TC
                nc.sync.dma_start(
                    a2a_in[h][bass.ds(g * (DH + 1), DH + 1), :],
                    cxl[:, bass.ts(half, TC)])

        def attn_block(h, b, filler=None):
            for ib in range(NI):
                av = avpsum.tile([128, IB], F32, tag="av", name="av")
                for jc in range(NJ):
                    attn_jc(h, b, ib, jc, av, filler)
                attn_finish_ib(h, b, ib, av)

        def fire_a2a(h):
            nc.gpsimd.collective_compute(
                "AllToAll",
                mybir.AluOpType.bypass,
                replica_groups=[list(range(NCORES))],
                ins=[a2a_in[h][:]],
                outs=[a2a_out[h][:]],
            )

        def gather_and_norm(hh):
            """a2a_out -> ctx_sb (skipping the denominator rows), then build
            1/l broadcast tiles and normalize ctx in place.  All DMAs on the
            sync queue; the anti-float markers emitted at phase-B end keep
            the Tile scheduler from hoisting these ahead of phase-B DMAs."""
            BRr = DH + 1
            for kc2 in range(NKC // HPC):
                kcp = hh * (NKC // HPC) + kc2
                for g2 in range(2):
                    nc.sync.dma_start(
                        ctx_sb[bass.ds(g2 * DH, DH), kcp, :],
                        a2a_out[hh][bass.ds((2 * kc2 + g2) * BRr, DH), :])
            # denominator rows, packed [8 groups x 8 col-chunks] on 64
            # partitions so the reciprocal's per-partition run is short
            for g in range(NCORES):
                nc.sync.dma_start(
                    ld64[bass.ds(g * 8, 8), hh, :],
                    a2a_out[hh][bass.ds(g * BRr + DH, 1), :]
                    .rearrange("o (c t) -> (o c) t", c=8))
            nc.vector.reciprocal(rl64[:, hh, :], ld64[:, hh, :])
            nc.sync.dma_start(
                r16d[hh].rearrange("g (c t) -> (g c) t", c=NCORES),
                rl64[:, hh, :])
            for kc2 in range(NKC // HPC):
                kcp = hh * (NKC // HPC) + kc2
                for g2 in range(2):
                    nc.sync.dma_start(
                        rl_sb[bass.ds(g2 * DH, DH), kcp, :],
                        r16d[hh, 2 * kc2 + g2:2 * kc2 + g2 + 1, :]
                        .to_broadcast((DH, TC)))
            hsl = bass.ds(hh * (NKC // HPC), NKC // HPC)
            nc.vector.tensor_mul(ctx_sb[:, hsl, :], ctx_sb[:, hsl, :],
                                 rl_sb[:, hsl, :])

        # ---------------- emission schedule ----------------
        # x DMAs all up-front (nothing compute-dependent ahead of them on
        # the sync queue); attention starts as soon as chunk 0's q/k are
        # projected, with every remaining projection unit spread through
        # the attention iterations as PE filler.
        from itertools import chain

        for i in range(NT):
            load_x_chunk(i)
        # wo/bo loads queue behind the x stream; needed only in phase C.
        nc.sync.dma_start(wo_sb[:], wo.rearrange("(kc p) n -> p kc n", p=128))
        nc.sync.dma_start(bo_sb[:], bo.to_broadcast((128, D)))

        proj_qk(0, "q")
        proj_qk(0, "k")
        # attn(0,0) ib0 over batch-0's first key half (chunk 0) starts
        # immediately; V(c0) + all of chunk 1 project inside it as filler.
        filler_a = chain(proj_units(0, "v"), proj_units(1, "kvq"))
        av00 = avpsum.tile([128, IB], F32, tag="av", name="av")
        for jc in range(NJ // 2):
            attn_jc(0, 0, 0, jc, av00, filler_a)
        for _ in filler_a:
            pass
        # second key half (chunk 1) + ib1; chunks 2,3 (batch 1) as filler.
        filler_b = chain(proj_units(2, "kvq"), proj_units(3, "kvq"))
        for jc in range(NJ // 2, NJ):
            attn_jc(0, 0, 0, jc, av00, filler_b)
        attn_finish_ib(0, 0, 0, av00)
        av01 = avpsum.tile([128, IB], F32, tag="av", name="av")
        for jc in range(NJ):
            attn_jc(0, 0, 1, jc, av01, filler_b)
        attn_finish_ib(0, 0, 1, av01)
        for _ in filler_b:
            pass
        # head-0 blocks first so A2A(0) fires at 50% of phase B and hides
        # under head-1 compute.
        attn_block(0, 1)           # needs chunks 2,3
        fire_a2a(0)
        attn_block(1, 0)
        attn_block(1, 1)
        # anti-float markers: tiny writes that complete at phase-B end and
        # make every post-A2A DMA's destination carry a WAW dependency, so
        # the Tile scheduler cannot hoist those DMAs (which wait on the
        # collectives) ahead of phase-B traffic in any engine queue.
        nc.vector.tensor_copy(ctx_sb[0:1, :, 0:1],
                              zero_col[0:1, :].to_broadcast((1, NKC, 1)))
        nc.vector.tensor_copy(rl_sb[0:1, :, 0:1],
                              zero_col[0:1, :].to_broadcast((1, NKC, 1)))
        nc.vector.tensor_copy(ld64[0:1, :, 0:1],
                              zero_col[0:1, :].to_broadcast((1, HPC, 1)))
        fire_a2a(1)

        abctx.close()

        # ---------------- phase C: output projection ----------------
        # Wo rows were permuted on the host so features of local head 0 of
        # all cores occupy chunks 0..3 and local head 1 chunks 4..7; the
        # head-0 half of the contraction runs while A2A(1) is in flight.
        opsum = ctx.enter_context(tc.tile_pool(name="opsum", bufs=1,
                                               space="PSUM"))
        ostage = ctx.enter_context(tc.tile_pool(name="ostage", bufs=4))

        pss = [opsum.tile([128, 512], F32, tag=f"ops{j}", name=f"ops{j}")
               for j in range(8)]

        for hh in range(HPC):
            gather_and_norm(hh)
            for kc2 in range(NKC // HPC):
                kcp = hh * (NKC // HPC) + kc2
                for tch in range(TC // 128):
                    for nh in range(2):
                        nc.tensor.matmul(
                            pss[tch * 2 + nh][:],
                            ctx_sb[:, kcp, bass.ts(tch, 128)],
                            wo_sb[:, kcp, bass.ts(nh, 512)],
                            start=(kcp == 0), stop=(kcp == NKC - 1),
                        )
        for tch in range(TC // 128):
            for nh in range(2):
                ot = ostage.tile([128, 512], F32, tag="ot", name="ot")
                nc.vector.tensor_add(ot[:], pss[tch * 2 + nh][:],
                                     bo_sb[:, bass.ts(nh, 512)])
                nc.sync.dma_start(out[bass.ts(tch, 128), bass.ts(nh, 512)],
                                  ot[:])


# ---------------- host-side sharding / unsharding ----------------

def rope_cos_sin_np(seq_len, d_head):
    inv_freq = 1.0 / (10000.0 ** (np.arange(0, d_head, 2, dtype=np.float32) / d_head))
    t = np.arange(seq_len, dtype=np.float32)
    freqs = np.einsum("i,j->ij", t, inv_freq).astype(np.float32)
    emb = np.concatenate((freqs, freqs), axis=-1)
    return np.cos(emb).astype(np.float32), np.sin(emb).astype(np.float32)


def _bf16(a):
    import ml_dtypes
    return np.ascontiguousarray(a.astype(ml_dtypes.bfloat16))


def make_in_maps(x, Wq, bq, Wk, bk, Wv, bv, Wo, bo):
    xT = _bf16(x.reshape(T, D).T)

    cos, sin = rope_cos_sin_np(L, DH)          # [L, 64]
    cosT = cos.T                               # [64, L]
    sinT = sin.T
    sgn = np.where(np.arange(DH) < DH // 2, -1.0, 1.0).astype(np.float32)
    sinT_signed = sinT * sgn[:, None]
    # stack 2 heads on partitions; batches reuse the same positions
    cosT_full = _bf16(np.tile(cosT, (HPC, 1)))      # [128, 2048]
    sinT_full = _bf16(np.tile(sinT_signed, (HPC, 1)))

    # permute Wo rows to local-head-major order: new row
    # f' = local_h*512 + core*64 + dh  <-  orig row core*128 + local_h*64 + dh
    perm = np.array([c * 128 + lh * 64 + dh
                     for lh in range(HPC)
                     for c in range(NCORES)
                     for dh in range(DH)], dtype=np.int64)
    wo_full = _bf16(Wo[perm, :])
    # bv folds into bo: softmax weights sum to 1, so the V bias passes
    # straight through attention and lands as bv @ Wo on every output row.
    bo_eff = np.ascontiguousarray(
        (bo + bv.astype(np.float64) @ Wo.astype(np.float64)).astype(np.float32)
        .reshape(1, D))

    in_maps = []
    for c in range(NCORES):
        sl = slice(c * 128, (c + 1) * 128)
        in_maps.append({
            "xT": xT,
            "wq": _bf16(Wq[:, sl]),
            "wk": _bf16(Wk[:, sl]),
            "wv": _bf16(Wv[:, sl]),
            "bq": np.ascontiguousarray(bq[sl].reshape(128, 1)),
            "bk": np.ascontiguousarray(bk[sl].reshape(128, 1)),
            "wo": wo_full,
            "bo": bo_eff,
            "cosT": cosT_full,
            "sinT": sinT_full,
        })
    return in_maps


def assemble_output(results):
    parts = [results[c]["out"] for c in range(NCORES)]
    return np.concatenate(parts, axis=0).reshape(B, L, D).astype(np.float32)


_CACHE = {}


def _get_runner():
    """Build the Bass program and a cached jitted SPMD executor once.

    Mirrors bass2jax.run_bass_via_pjrt's multi-core path, but keeps the
    jitted shard_map callable alive so repeat kernel() calls skip retracing.
    """
    if "runner" in _CACHE:
        return _CACHE["runner"]

    import jax
    import numpy as _np
    from jax.sharding import Mesh, PartitionSpec
    from jax.experimental.shard_map import shard_map
    from concourse import bass2jax, mybir as _mybir

    nc = build_nc()
    bass2jax.install_neuronx_cc_hook()

    partition_name = (nc.partition_id_tensor.name
                      if nc.partition_id_tensor else None)
    in_names, out_names, out_avals, zero_shapes = [], [], [], []
    for alloc in nc.m.functions[0].allocations:
        if not isinstance(alloc, _mybir.MemoryLocationSet):
            continue
        name = alloc.memorylocations[0].name
        if alloc.kind == "ExternalInput":
            if name != partition_name:
                in_names.append(name)
        elif alloc.kind == "ExternalOutput":
            shape = tuple(alloc.tensor_shape)
            dtype = _mybir.dt.np(alloc.dtype)
            out_names.append(name)
            out_avals.append(jax.core.ShapedArray(shape, dtype))
            zero_shapes.append((shape, dtype))
    n_params = len(in_names)
    n_outs = len(out_avals)
    all_in_names = list(in_names) + list(out_names)
    if partition_name is not None:
        all_in_names.append(partition_name)
    donate = tuple(range(n_params, n_params + n_outs))

    def _body(*args):
        operands = list(args)
        if partition_name is not None:
            operands.append(bass2jax.partition_id_tensor())
        outs = bass2jax._bass_exec_p.bind(
            *operands,
            out_avals=tuple(out_avals),
            in_names=tuple(all_in_names),
            out_names=tuple(out_names),
            lowering_input_output_aliases=(),
            sim_require_finite=True,
            sim_require_nnan=True,
            nc=nc,
        )
        return tuple(outs)

    devices = jax.devices()[:NCORES]
    mesh = Mesh(_np.asarray(devices), ("core",))
    in_specs = (PartitionSpec("core"),) * (n_params + n_outs)
    out_specs = (PartitionSpec("core"),) * n_outs
    sharded = jax.jit(
        shard_map(_body, mesh=mesh, in_specs=in_specs, out_specs=out_specs,
                  check_rep=False),
        donate_argnums=donate,
        keep_unused=True,
    )

    def run(in_maps):
        per_core = [[_np.asarray(m[name]) for name in in_names]
                    for m in in_maps]
        concat_in = [
            _np.concatenate([per_core[c][i] for c in range(NCORES)], axis=0)
            for i in range(n_params)
        ]
        concat_zeros = [
            _np.zeros((NCORES * s[0], *s[1:]), dt) for s, dt in zero_shapes
        ]
        out_arrs = sharded(*concat_in, *concat_zeros)
        return [
            {name: _np.asarray(out_arrs[i]).reshape(
                NCORES, *out_avals[i].shape)[c]
             for i, name in enumerate(out_names)}
            for c in range(NCORES)
        ]

    _CACHE["runner"] = run
    return run


def kernel(**inputs):
    run = _get_runner()
    in_maps = make_in_maps(**{k: np.asarray(v, dtype=np.float32)
                              for k, v in inputs.items()})
    return assemble_output(run(in_maps))
